# revision 18
# baseline (speedup 1.0000x reference)
"""Trainium2 Bass kernel for nn_Damping: per-sample Cholesky-factor damping.

Math (per sample b):
  h  = tanh MLPs of x0 -> diag xd [64], offdiag z [2016] (strict lower tri of L)
  y  = L^T x0 ; D = L y

Implementation (per core, feature-major layout [feature partitions, batch free]):
  - all matmuls in bf16 (PE streams 2 elem/cycle/partition -> ~1.4x over
    fp32r at free=512); validated l2 error ~5e-3 vs the 2e-2 gate
  - L matvecs without materializing L, via static 0/1 scatter/gather
    matrices on the tensor engine:
      x0g = R @ x0            (x0g[p] = x0[row(p)])
      u   = z * x0g           (DVE: z f32 SBUF x x0g f32 PSUM -> u bf16;
                               the only fast PSUM-operand path)
      y   = C^T u + xd*x0 + bdo*x0   (scatters accumulate in PSUM; the
            diag terms enter as one packed pair of K=64 matmuls)
      yg  = C @ y ; D = R^T (z*yg) + xd*y + bdo*y
    boo is folded into the z PSUM->SBUF copy as an activation bias
    (Identity+bias, f32 out); bdo via a diag(bdo) matmul packed with the
    identity diag matmul at tile_position (0,0)/(64,0).
  - K=64 matmul pairs (gathers, h1 o/d, diag+bdo) are emitted adjacently at
    row groups 0/64 so both stream concurrently through the PE array.
  - flat per-tile emission; the Tile scheduler's ready+priority order lets
    tile t+1's dense MLP/z matmuls fill the PE while tile t's matvec
    latency chains wait on DVE.

Data parallel over 8 cores: batch 32768 -> 8 x 4096.
"""

import sys

if "/opt/trn_rl_repo" not in sys.path:
    sys.path.insert(0, "/opt/trn_rl_repo")

import numpy as np

N = 64
H = 256
B = 32768
OFF = 2016
NCORES = 8
B_CORE = B // NCORES   # 4096
F = 512                # batch tile (free dim)
NCHUNK = 16            # 2016 = 16 * 126
CH = OFF // NCHUNK     # 126


def _build_nc(b_core=B_CORE, f=F, reps=1):
    """reps>1 unrolls the whole batch loop `reps` times inside one NEFF
    (same data, same outputs) — used by the timing harness to measure
    steady-state per-pass device time without dispatch overhead."""
    import concourse.bacc as bacc
    import concourse.mybir as mybir
    import concourse.tile as tile

    F32 = mybir.dt.float32
    F32R = mybir.dt.float32r
    BF16 = mybir.dt.bfloat16
    Tanh = mybir.ActivationFunctionType.Tanh
    Copy = mybir.ActivationFunctionType.Copy
    Ident = mybir.ActivationFunctionType.Identity

    ntiles = b_core // f
    assert b_core % f == 0 and f % 128 == 0
    ncol = f // 128

    nc = bacc.Bacc("TRN2", target_bir_lowering=False, debug=False,
                   num_devices=NCORES)

    # --- DRAM tensors ---
    x_d = nc.dram_tensor("x", [b_core, N], F32R, kind="ExternalInput")
    # w1pk: wo1^T on partitions 0..63, wd1^T on 64..127 (h1 row-pair packing)
    w1_d = nc.dram_tensor("w1pk", [128, H], BF16, kind="ExternalInput")
    wd2_d = nc.dram_tensor("wd2t", [H, H], BF16, kind="ExternalInput")
    wdo_d = nc.dram_tensor("wdot", [H, N], BF16, kind="ExternalInput")
    wo2_d = nc.dram_tensor("wo2t", [H, H], BF16, kind="ExternalInput")
    woo_d = nc.dram_tensor("woot", [H, OFF], BF16, kind="ExternalInput")
    # gather lhsT matrices, duplicated on both partition halves for 2x
    # row-group packing (tile_position (0,0) / (64,0))
    rt_d = nc.dram_tensor("rtmat", [128, OFF], BF16, kind="ExternalInput")
    ct_d = nc.dram_tensor("ctmat", [128, OFF], BF16, kind="ExternalInput")
    # scatter lhsT chunks packed columnwise: col block m = R/C[m*CH:(m+1)*CH]
    r_d = nc.dram_tensor("rmpk", [CH, NCHUNK * N], BF16, kind="ExternalInput")
    c_d = nc.dram_tensor("cmpk", [CH, NCHUNK * N], BF16, kind="ExternalInput")
    id_d = nc.dram_tensor("ident", [128, 128], F32R, kind="ExternalInput")
    idb_d = nc.dram_tensor("identb", [N, N], BF16, kind="ExternalInput")
    qd_d = nc.dram_tensor("qdm", [N, N], BF16, kind="ExternalInput")
    # tanh biases packed: cols 0,1=bo1 k0/k1; 2,3=bd1; 4,5=bo2; 6,7=bd2
    bt_d = nc.dram_tensor("btab", [128, 8], F32, kind="ExternalInput")
    boo_d = nc.dram_tensor("booc", [CH, NCHUNK], F32, kind="ExternalInput")
    out_d = nc.dram_tensor("out", [b_core, N], F32, kind="ExternalOutput")

    with tile.TileContext(nc) as tc:
        with (
            tc.tile_pool(name="wpool", bufs=1) as wp,
            tc.tile_pool(name="apool", bufs=1) as ap,
            tc.tile_pool(name="zpool", bufs=1) as zp,
            tc.tile_pool(name="upool", bufs=1) as up,
            tc.tile_pool(name="iopool", bufs=1) as iop,
            tc.tile_pool(name="psum", bufs=1, space="PSUM") as pp,
        ):
            def wtile(name, src, shape, dt=BF16, eng=nc.sync):
                t = wp.tile(shape, dt, tag=name, name=name, bufs=1)
                eng.dma_start(t[:], src)
                return t

            # light weights first on the sync queue (unblock tile 0)
            ident = wtile("ident", id_d[:], [128, 128], F32R)
            identb = wtile("identb", idb_d[:], [N, N])
            qdm = wtile("qdm", qd_d[:], [N, N])
            w1 = wtile("w1", w1_d[:], [128, H])
            btab = wtile("btab", bt_d[:], [128, 8], F32)
            boo = wtile("boo", boo_d[:], [CH, NCHUNK], F32)
            bo1 = [btab[:, k:k + 1] for k in (0, 1)]
            bd1 = [btab[:, k:k + 1] for k in (2, 3)]
            bo2 = [btab[:, k:k + 1] for k in (4, 5)]
            bd2 = [btab[:, k:k + 1] for k in (6, 7)]

            # x-input DMA queue (sync ring, ahead of the heavy weights)
            total_tiles = ntiles * reps
            x_tiles = {}

            def emit_xdma(tt):
                if tt >= total_tiles:
                    return
                t = tt % ntiles
                xt = iop.tile([128, ncol, N], F32R, tag="x_in", bufs=3,
                              name=f"x_in{tt}")
                nc.sync.dma_start(
                    xt[:], x_d[t * f:(t + 1) * f, :]
                    .rearrange("(c p) n -> p c n", p=128))
                x_tiles[tt] = xt

            emit_xdma(0)
            emit_xdma(1)

            # heavy weights on the gpsimd (SWDGE) queue: doesn't consume the
            # sync/Act engine streams during the prologue
            wd2 = [wtile(f"wd2_{k}", wd2_d[k * 128:(k + 1) * 128, :], [128, H],
                         eng=nc.gpsimd) for k in range(2)]
            wo2 = [wtile(f"wo2_{k}", wo2_d[k * 128:(k + 1) * 128, :], [128, H],
                         eng=nc.gpsimd) for k in range(2)]
            wdo = [wtile(f"wdo_{k}", wdo_d[k * 128:(k + 1) * 128, :], [128, N],
                         eng=nc.gpsimd) for k in range(2)]
            woo = [wtile(f"woo_{k}", woo_d[k * 128:(k + 1) * 128, :],
                         [128, OFF], eng=nc.gpsimd) for k in range(2)]
            rtm = wtile("rtm", rt_d[:], [128, OFF], eng=nc.gpsimd)
            ctm = wtile("ctm", ct_d[:], [128, OFF], eng=nc.gpsimd)
            rmat = wtile("rmat", r_d[:], [CH, NCHUNK * N], eng=nc.gpsimd)
            cmat = wtile("cmat", c_d[:], [CH, NCHUNK * N], eng=nc.gpsimd)

            def emit_mlp(tt):
                """Input transposes, MLPs, z production for tile tt."""
                st = {}
                xt = x_tiles.pop(tt)
                emit_xdma(tt + 2)
                t = f"r{tt}"
                # input transpose via PE (ph ring slot), then copy + dup
                px = pp.tile([N, ncol, 128], F32R, tag="ph", bufs=2,
                             name=f"px{t}")
                for c in range(ncol):
                    nc.tensor.transpose(px[:, c, :], xt[:, c, :], ident[:])
                x0T = ap.tile([128, f], BF16, tag="x0T", bufs=2, name=f"x0T{t}")
                pxf = px.rearrange("p c n -> p (c n)")
                nc.scalar.activation(x0T[0:N, :], pxf, Copy)
                nc.sync.dma_start(x0T[N:2 * N, :], x0T[0:N, :])
                st["x0T"] = x0T

                # h1: o/d packed as row-group pairs (wo1 rows 0-63 with
                # x0T lo half, wd1 rows 64-127 with the dup'd hi half)
                h1o, h1d = [], []
                for m in range(2):
                    phs = []
                    for i, lo in ((0, 0), (1, N)):
                        ph = pp.tile([128, f], F32, tag="ph", bufs=2,
                                     name=f"ph_h1_{m}{i}_{t}")
                        nc.tensor.matmul(
                            ph[:], w1[lo:lo + N, m * 128:(m + 1) * 128],
                            x0T[lo:lo + N, :], start=True, stop=True,
                            tile_position=(lo, 0))
                        phs.append(ph)
                    for i, (outs, biases) in enumerate(
                            ((h1o, bo1), (h1d, bd1))):
                        h = ap.tile([128, f], BF16, tag=f"h1_{i}{m}", bufs=2,
                                    name=f"h1_{i}{m}_{t}")
                        nc.scalar.activation(h[:], phs[i][:], Tanh,
                                             bias=biases[m])
                        outs.append(h)

                def layer2(tag, wts, rhss, biases):
                    outs = []
                    for m in range(2):
                        ph = pp.tile([128, f], F32, tag="ph", bufs=2,
                                     name=f"ph_{tag}{m}_{t}")
                        for k in range(2):
                            nc.tensor.matmul(
                                ph[:], wts[k][:, m * 128:(m + 1) * 128],
                                rhss[k][:, :], start=(k == 0), stop=(k == 1))
                        h = ap.tile([128, f], BF16, tag=f"{tag}{m}", bufs=2,
                                    name=f"{tag}{m}_{t}")
                        nc.scalar.activation(h[:], ph[:], Tanh, bias=biases[m])
                        outs.append(h)
                    return outs

                h2o = layer2("h2o", wo2, h1o, bo2)
                h2d = layer2("h2d", wd2, h1d, bd2)

                # z chunks (longest pole), diag head after. Chunk PAIRS
                # share one [CH, 2f] SBUF tile so the matvec can multiply
                # both chunks in a single wide DVE op.
                z_sb = []
                for mp in range(0, NCHUNK, 2):
                    z2 = zp.tile([CH, 2 * f], F32, tag=f"z2_{mp}", bufs=2,
                                 name=f"z2_{mp}_{t}")
                    for i in (0, 1):
                        m = mp + i
                        pz = pp.tile([CH, f], F32, tag="ph", bufs=2,
                                     name=f"pz{m}_{t}")
                        for k in range(2):
                            nc.tensor.matmul(
                                pz[:], woo[k][:, m * CH:(m + 1) * CH],
                                h2o[k][:], start=(k == 0), stop=(k == 1))
                        # fold boo in as the activation bias (f32 out keeps
                        # the DVE PSUM-mul on its fast f32xf32 path)
                        nc.scalar.activation(z2[:, i * f:(i + 1) * f], pz[:],
                                             Ident, bias=boo[:, m:m + 1])
                    z_sb.append(z2)
                st["z"] = z_sb

                pxd = pp.tile([N, f], F32, tag="ph", bufs=2, name=f"pxd{t}")
                for k in range(2):
                    nc.tensor.matmul(pxd[:], wdo[k][:], h2d[k][:],
                                     start=(k == 0), stop=(k == 1))
                # xd WITHOUT bdo (bdo enters via the diag(bdo) matmul)
                xd = ap.tile([N, f], BF16, tag="xd", bufs=2, name=f"xd{t}")
                nc.scalar.activation(xd[:], pxd[:], Copy)
                st["xd"] = xd
                return st

            def emit_matvec(t, rhs, z_sb, gather_w, scatter_w, diag_t, name):
                """acc = scatter_w^T (z * gather_w@rhs) + diag_t + bdo*rhs.
                Gather pairs emitted adjacently at row groups 0/64; scatters
                accumulate into one PSUM bank; the diag identity matmul and
                the diag(bdo) matmul close the group as one packed pair."""
                acc = pp.tile([N, f], F32, tag="acc", bufs=2,
                              name=f"p{name}{t}")
                for pi, mp in enumerate(range(0, NCHUNK, 2)):
                    # gather pair into one 2-bank PSUM tile; both halves
                    # free together so the next pair issues adjacently and
                    # the row-group matmuls overlap
                    pg2 = pp.tile([CH, 2 * f], F32, tag="pg", bufs=2,
                                  name=f"pg2_{name}{mp}_{t}")
                    for i in (0, 1):
                        lo = i * N  # even chunk -> lo half, odd -> hi half
                        nc.tensor.matmul(
                            pg2[:, i * f:(i + 1) * f],
                            gather_w[lo:lo + N, (mp + i) * CH:(mp + i + 1) * CH],
                            rhs[lo:lo + N, :], start=True, stop=True,
                            tile_position=(lo, 0))
                    u2 = up.tile([CH, 2 * f], BF16, tag="u", bufs=9,
                                 name=f"u2_{name}{mp}_{t}")
                    nc.vector.tensor_mul(u2[:], z_sb[pi][:], pg2[:])
                    for i in (0, 1):
                        nc.tensor.matmul(
                            acc[:], scatter_w[:, (mp + i) * N:(mp + i + 1) * N],
                            u2[:, i * f:(i + 1) * f],
                            start=(mp == 0 and i == 0), stop=False,
                            skip_group_check=True)
                # diag identity term + diag(bdo) term close the group
                # (both at row group 0 — accumulating from tile_position
                # (64,0) into an open group faults on HW)
                nc.tensor.matmul(acc[:], identb[:], diag_t[:],
                                 start=False, stop=False,
                                 skip_group_check=True)
                nc.tensor.matmul(acc[:], qdm[:], rhs[0:N, :],
                                 start=False, stop=True,
                                 skip_group_check=True)
                return acc

            def emit_tail(tt, st):
                """Both L matvecs + output for tile tt."""
                t_out = tt % ntiles
                t = f"r{tt}"
                x0T, z_sb, xd = st["x0T"], st["z"], st["xd"]
                t1 = ap.tile([N, f], BF16, tag="t1", bufs=2, name=f"t1_{t}")
                nc.vector.tensor_mul(t1[:], xd[:], x0T[0:N, :])

                py = emit_matvec(t, x0T, z_sb, rtm, cmat, t1, "y")
                y = ap.tile([128, f], BF16, tag="y", bufs=2, name=f"y{t}")
                nc.scalar.activation(y[0:N, :], py[:], Copy)
                nc.sync.dma_start(y[N:2 * N, :], y[0:N, :])
                t2 = ap.tile([N, f], BF16, tag="t2", bufs=2, name=f"t2_{t}")
                nc.vector.tensor_mul(t2[:], xd[:], y[0:N, :])

                pd = emit_matvec(t, y, z_sb, ctm, rmat, t2, "d")
                dd = ap.tile([N, f], F32R, tag="dd", bufs=2, name=f"dd{t}")
                nc.scalar.activation(dd[:], pd[:], Copy)

                # output transpose via PE into the acc ring, copy, DMA out
                po = pp.tile([128, ncol, N], F32R, tag="acc", bufs=2,
                             name=f"po{t}")
                for c in range(ncol):
                    nc.tensor.transpose(po[:, c, :],
                                        dd[:, c * 128:(c + 1) * 128],
                                        ident[:N, :N])
                o_sb = iop.tile([128, ncol, N], F32, tag="o_sb", bufs=2,
                                name=f"o_sb{t}")
                nc.scalar.activation(o_sb[:], po.rearrange("p c n -> p (c n)"),
                                     Copy)
                nc.gpsimd.dma_start(
                    out_d[t_out * f:(t_out + 1) * f, :]
                    .rearrange("(c p) n -> p c n", p=128), o_sb[:])

            # ---- flat per-tile emission ----
            for tt in range(total_tiles):
                st = emit_mlp(tt)
                emit_tail(tt, st)

    nc.compile()
    return nc


def _host_constants(Wd1, bd1, Wd2, bd2, Wdo, bdo, Wo1, bo1, Wo2, bo2, Woo, boo):
    """Shared (per-core replicated) input arrays."""
    import ml_dtypes
    f32 = np.float32
    bf16 = ml_dtypes.bfloat16
    rows, cols = np.tril_indices(N, k=-1)
    R = np.zeros((OFF, N), f32)
    R[np.arange(OFF), rows] = 1.0
    C = np.zeros((OFF, N), f32)
    C[np.arange(OFF), cols] = 1.0

    def bt(a):
        return np.ascontiguousarray(np.asarray(a, f32)).astype(bf16)

    def ct(a):
        return np.ascontiguousarray(a, dtype=f32)

    # packed scatter chunks: col block m = chunk m ([CH, N] each)
    rmpk = np.concatenate([R[m * CH:(m + 1) * CH] for m in range(NCHUNK)],
                          axis=1)
    cmpk = np.concatenate([C[m * CH:(m + 1) * CH] for m in range(NCHUNK)],
                          axis=1)
    btab = np.stack([np.asarray(b, f32).reshape(-1)
                     for b in (bo1[:128], bo1[128:], bd1[:128], bd1[128:],
                               bo2[:128], bo2[128:], bd2[:128], bd2[128:])],
                    axis=1)
    w1pk = np.vstack([np.asarray(Wo1, f32).T[:N], np.asarray(Wd1, f32).T[:N]])

    return {
        "w1pk": bt(w1pk), "wd2t": bt(np.asarray(Wd2).T),
        "wdot": bt(np.asarray(Wdo).T),
        "wo2t": bt(np.asarray(Wo2).T), "woot": bt(np.asarray(Woo).T),
        "rtmat": bt(np.vstack([R.T, R.T])), "ctmat": bt(np.vstack([C.T, C.T])),
        "rmpk": bt(rmpk), "cmpk": bt(cmpk),
        "ident": np.eye(128, dtype=f32),
        "identb": bt(np.eye(N, dtype=f32)),
        "qdm": bt(np.diag(np.asarray(bdo, f32))),
        "btab": ct(btab),
        "booc": ct(np.asarray(boo, f32).reshape(NCHUNK, CH).T),
    }


_NC_CACHE = {}


def get_nc(b_core=B_CORE, f=F, reps=1):
    key = (b_core, f, reps)
    if key not in _NC_CACHE:
        _NC_CACHE[key] = _build_nc(b_core, f, reps)
    return _NC_CACHE[key]


def make_in_maps(input, **params):
    shared = _host_constants(**params)
    x = np.ascontiguousarray(np.asarray(input), dtype=np.float32)
    assert x.shape == (B, N)
    return [dict(shared, x=x[c * B_CORE:(c + 1) * B_CORE]) for c in range(NCORES)]


def kernel(input, **params):
    from concourse import bass_utils

    nc = get_nc()
    in_maps = make_in_maps(input, **params)
    res = bass_utils.run_bass_kernel_spmd(nc, in_maps,
                                          core_ids=list(range(NCORES)))
    return np.concatenate([r["out"] for r in res.results], axis=0)


# revision 22
# speedup vs baseline: 1.0119x; 1.0119x over previous
"""Trainium2 Bass kernel for nn_Damping: per-sample Cholesky-factor damping.

Math (per sample b):
  h  = tanh MLPs of x0 -> diag xd [64], offdiag z [2016] (strict lower tri of L)
  y  = L^T x0 ; D = L y

Implementation (per core, feature-major layout [feature partitions, batch free]):
  - all matmuls in bf16 (PE streams 2 elem/cycle/partition -> ~1.4x over
    fp32r at free=512); validated l2 error ~5e-3 vs the 2e-2 gate
  - L matvecs without materializing L, via static 0/1 scatter/gather
    matrices on the tensor engine:
      x0g = R @ x0            (x0g[p] = x0[row(p)])
      u   = z * x0g           (DVE: z f32 SBUF x x0g f32 PSUM -> u bf16;
                               the only fast PSUM-operand path)
      y   = C^T u + xd*x0 + bdo*x0   (scatters accumulate in PSUM; the
            diag terms enter as one packed pair of K=64 matmuls)
      yg  = C @ y ; D = R^T (z*yg) + xd*y + bdo*y
    boo is folded into the z PSUM->SBUF copy as an activation bias
    (Identity+bias, f32 out); bdo via a diag(bdo) matmul packed with the
    identity diag matmul at tile_position (0,0)/(64,0).
  - K=64 matmul pairs (gathers, h1 o/d, diag+bdo) are emitted adjacently at
    row groups 0/64 so both stream concurrently through the PE array.
  - flat per-tile emission; the Tile scheduler's ready+priority order lets
    tile t+1's dense MLP/z matmuls fill the PE while tile t's matvec
    latency chains wait on DVE.

Data parallel over 8 cores: batch 32768 -> 8 x 4096.
"""

import sys

if "/opt/trn_rl_repo" not in sys.path:
    sys.path.insert(0, "/opt/trn_rl_repo")

import numpy as np

N = 64
H = 256
B = 32768
OFF = 2016
NCORES = 8
B_CORE = B // NCORES   # 4096
F = 512                # batch tile (free dim)
NCHUNK = 16            # 2016 = 16 * 126
CH = OFF // NCHUNK     # 126


def _build_nc(b_core=B_CORE, f=F, reps=1):
    """reps>1 unrolls the whole batch loop `reps` times inside one NEFF
    (same data, same outputs) — used by the timing harness to measure
    steady-state per-pass device time without dispatch overhead."""
    import concourse.bacc as bacc
    import concourse.mybir as mybir
    import concourse.tile as tile

    F32 = mybir.dt.float32
    F32R = mybir.dt.float32r
    BF16 = mybir.dt.bfloat16
    Tanh = mybir.ActivationFunctionType.Tanh
    Copy = mybir.ActivationFunctionType.Copy
    Ident = mybir.ActivationFunctionType.Identity

    ntiles = b_core // f
    assert b_core % f == 0 and f % 128 == 0
    ncol = f // 128

    nc = bacc.Bacc("TRN2", target_bir_lowering=False, debug=False,
                   num_devices=NCORES)

    # --- DRAM tensors ---
    x_d = nc.dram_tensor("x", [b_core, N], F32R, kind="ExternalInput")
    # w1pk: wo1^T on partitions 0..63, wd1^T on 64..127 (h1 row-pair packing)
    w1_d = nc.dram_tensor("w1pk", [128, H], BF16, kind="ExternalInput")
    wd2_d = nc.dram_tensor("wd2t", [H, H], BF16, kind="ExternalInput")
    wdo_d = nc.dram_tensor("wdot", [H, N], BF16, kind="ExternalInput")
    wo2_d = nc.dram_tensor("wo2t", [H, H], BF16, kind="ExternalInput")
    woo_d = nc.dram_tensor("woot", [H, OFF], BF16, kind="ExternalInput")
    # gather lhsT matrices, duplicated on both partition halves for 2x
    # row-group packing (tile_position (0,0) / (64,0))
    rt_d = nc.dram_tensor("rtmat", [128, OFF], BF16, kind="ExternalInput")
    ct_d = nc.dram_tensor("ctmat", [128, OFF], BF16, kind="ExternalInput")
    # scatter lhsT chunks packed columnwise: col block m = R/C[m*CH:(m+1)*CH]
    r_d = nc.dram_tensor("rmpk", [CH, NCHUNK * N], BF16, kind="ExternalInput")
    c_d = nc.dram_tensor("cmpk", [CH, NCHUNK * N], BF16, kind="ExternalInput")
    id_d = nc.dram_tensor("ident", [128, 128], F32R, kind="ExternalInput")
    idb_d = nc.dram_tensor("identb", [N, N], BF16, kind="ExternalInput")
    qd_d = nc.dram_tensor("qdm", [N, N], BF16, kind="ExternalInput")
    # tanh biases packed: cols 0,1=bo1 k0/k1; 2,3=bd1; 4,5=bo2; 6,7=bd2
    bt_d = nc.dram_tensor("btab", [128, 8], F32, kind="ExternalInput")
    boo_d = nc.dram_tensor("booc", [CH, NCHUNK], F32, kind="ExternalInput")
    out_d = nc.dram_tensor("out", [b_core, N], F32, kind="ExternalOutput")

    with tile.TileContext(nc) as tc:
        with (
            tc.tile_pool(name="wpool", bufs=1) as wp,
            tc.tile_pool(name="apool", bufs=1) as ap,
            tc.tile_pool(name="zpool", bufs=1) as zp,
            tc.tile_pool(name="upool", bufs=1) as up,
            tc.tile_pool(name="iopool", bufs=1) as iop,
            tc.tile_pool(name="psum", bufs=1, space="PSUM") as pp,
        ):
            def wtile(name, src, shape, dt=BF16, eng=nc.sync):
                t = wp.tile(shape, dt, tag=name, name=name, bufs=1)
                eng.dma_start(t[:], src)
                return t

            # light weights first on the sync queue (unblock tile 0)
            ident = wtile("ident", id_d[:], [128, 128], F32R)
            identb = wtile("identb", idb_d[:], [N, N])
            qdm = wtile("qdm", qd_d[:], [N, N])
            w1 = wtile("w1", w1_d[:], [128, H])
            btab = wtile("btab", bt_d[:], [128, 8], F32)
            boo = wtile("boo", boo_d[:], [CH, NCHUNK], F32)
            bo1 = [btab[:, k:k + 1] for k in (0, 1)]
            bd1 = [btab[:, k:k + 1] for k in (2, 3)]
            bo2 = [btab[:, k:k + 1] for k in (4, 5)]
            bd2 = [btab[:, k:k + 1] for k in (6, 7)]

            # x-input DMA queue (sync ring, ahead of the heavy weights)
            total_tiles = ntiles * reps
            x_tiles = {}

            def emit_xdma(tt):
                if tt >= total_tiles:
                    return
                t = tt % ntiles
                xt = iop.tile([128, ncol, N], F32R, tag="x_in", bufs=3,
                              name=f"x_in{tt}")
                nc.sync.dma_start(
                    xt[:], x_d[t * f:(t + 1) * f, :]
                    .rearrange("(c p) n -> p c n", p=128))
                x_tiles[tt] = xt

            emit_xdma(0)
            emit_xdma(1)

            # heavy weights on the gpsimd (SWDGE) queue: doesn't consume the
            # sync/Act engine streams during the prologue
            wd2 = [wtile(f"wd2_{k}", wd2_d[k * 128:(k + 1) * 128, :], [128, H],
                         eng=nc.gpsimd) for k in range(2)]
            wo2 = [wtile(f"wo2_{k}", wo2_d[k * 128:(k + 1) * 128, :], [128, H],
                         eng=nc.gpsimd) for k in range(2)]
            wdo = [wtile(f"wdo_{k}", wdo_d[k * 128:(k + 1) * 128, :], [128, N],
                         eng=nc.gpsimd) for k in range(2)]
            woo = [wtile(f"woo_{k}", woo_d[k * 128:(k + 1) * 128, :],
                         [128, OFF], eng=nc.gpsimd) for k in range(2)]
            rtm = wtile("rtm", rt_d[:], [128, OFF], eng=nc.gpsimd)
            ctm = wtile("ctm", ct_d[:], [128, OFF], eng=nc.gpsimd)
            rmat = wtile("rmat", r_d[:], [CH, NCHUNK * N], eng=nc.gpsimd)
            cmat = wtile("cmat", c_d[:], [CH, NCHUNK * N], eng=nc.gpsimd)

            def emit_mlp(tt):
                """Input transposes, MLPs, z production for tile tt."""
                st = {}
                xt = x_tiles.pop(tt)
                emit_xdma(tt + 2)
                t = f"r{tt}"
                # input transpose via PE (ph ring slot), then copy + dup
                px = pp.tile([N, ncol, 128], F32R, tag="ph", bufs=2,
                             name=f"px{t}")
                for c in range(ncol):
                    nc.tensor.transpose(px[:, c, :], xt[:, c, :], ident[:])
                x0T = ap.tile([128, f], BF16, tag="x0T", bufs=2, name=f"x0T{t}")
                pxf = px.rearrange("p c n -> p (c n)")
                nc.scalar.activation(x0T[0:N, :], pxf, Copy)
                nc.sync.dma_start(x0T[N:2 * N, :], x0T[0:N, :])
                st["x0T"] = x0T

                # h1: o/d packed as row-group pairs (wo1 rows 0-63 with
                # x0T lo half, wd1 rows 64-127 with the dup'd hi half)
                h1o, h1d = [], []
                for m in range(2):
                    phs = []
                    for i, lo in ((0, 0), (1, N)):
                        ph = pp.tile([128, f], F32, tag="ph", bufs=2,
                                     name=f"ph_h1_{m}{i}_{t}")
                        nc.tensor.matmul(
                            ph[:], w1[lo:lo + N, m * 128:(m + 1) * 128],
                            x0T[lo:lo + N, :], start=True, stop=True,
                            tile_position=(lo, 0))
                        phs.append(ph)
                    for i, (outs, biases) in enumerate(
                            ((h1o, bo1), (h1d, bd1))):
                        h = ap.tile([128, f], BF16, tag=f"h1_{i}{m}", bufs=2,
                                    name=f"h1_{i}{m}_{t}")
                        nc.scalar.activation(h[:], phs[i][:], Tanh,
                                             bias=biases[m])
                        outs.append(h)

                def layer2(tag, wts, rhss, biases):
                    outs = []
                    for m in range(2):
                        ph = pp.tile([128, f], F32, tag="ph", bufs=2,
                                     name=f"ph_{tag}{m}_{t}")
                        for k in range(2):
                            nc.tensor.matmul(
                                ph[:], wts[k][:, m * 128:(m + 1) * 128],
                                rhss[k][:, :], start=(k == 0), stop=(k == 1))
                        h = ap.tile([128, f], BF16, tag=f"{tag}{m}", bufs=2,
                                    name=f"{tag}{m}_{t}")
                        nc.scalar.activation(h[:], ph[:], Tanh, bias=biases[m])
                        outs.append(h)
                    return outs

                h2o = layer2("h2o", wo2, h1o, bo2)
                h2d = layer2("h2d", wd2, h1d, bd2)

                # z chunks (longest pole), diag head after. Chunk PAIRS
                # share one [CH, 2f] SBUF tile so the matvec can multiply
                # both chunks in a single wide DVE op.
                z_sb = []
                for mp in range(0, NCHUNK, 2):
                    z2 = zp.tile([CH, 2 * f], F32, tag=f"z2_{mp}", bufs=2,
                                 name=f"z2_{mp}_{t}")
                    for i in (0, 1):
                        m = mp + i
                        pz = pp.tile([CH, f], F32, tag="ph", bufs=2,
                                     name=f"pz{m}_{t}")
                        for k in range(2):
                            nc.tensor.matmul(
                                pz[:], woo[k][:, m * CH:(m + 1) * CH],
                                h2o[k][:], start=(k == 0), stop=(k == 1))
                        # fold boo in as the activation bias (f32 out keeps
                        # the DVE PSUM-mul on its fast f32xf32 path)
                        nc.scalar.activation(z2[:, i * f:(i + 1) * f], pz[:],
                                             Ident, bias=boo[:, m:m + 1])
                    z_sb.append(z2)
                st["z"] = z_sb

                pxd = pp.tile([N, f], F32, tag="ph", bufs=2, name=f"pxd{t}")
                for k in range(2):
                    nc.tensor.matmul(pxd[:], wdo[k][:], h2d[k][:],
                                     start=(k == 0), stop=(k == 1))
                # xd WITHOUT bdo (bdo enters via the diag(bdo) matmul)
                xd = ap.tile([N, f], BF16, tag="xd", bufs=2, name=f"xd{t}")
                nc.scalar.activation(xd[:], pxd[:], Copy)
                st["xd"] = xd
                return st

            def emit_matvec(t, rhs, z_sb, gather_w, scatter_w, diag_t, name):
                """acc = scatter_w^T (z * gather_w@rhs) + diag_t + bdo*rhs.
                Gather pairs emitted adjacently at row groups 0/64; scatters
                accumulate into one PSUM bank; the diag identity matmul and
                the diag(bdo) matmul close the group as one packed pair."""
                acc = pp.tile([N, f], F32, tag="acc", bufs=2,
                              name=f"p{name}{t}")
                for pi, mp in enumerate(range(0, NCHUNK, 2)):
                    # gather pair into one 2-bank PSUM tile; both halves
                    # free together so the next pair issues adjacently and
                    # the row-group matmuls overlap
                    pg2 = pp.tile([CH, 2 * f], F32, tag="pg", bufs=2,
                                  name=f"pg2_{name}{mp}_{t}")
                    for i in (0, 1):
                        lo = i * N  # even chunk -> lo half, odd -> hi half
                        nc.tensor.matmul(
                            pg2[:, i * f:(i + 1) * f],
                            gather_w[lo:lo + N, (mp + i) * CH:(mp + i + 1) * CH],
                            rhs[lo:lo + N, :], start=True, stop=True,
                            tile_position=(lo, 0))
                    u2 = up.tile([CH, 2 * f], BF16, tag="u", bufs=9,
                                 name=f"u2_{name}{mp}_{t}")
                    nc.vector.tensor_mul(u2[:], z_sb[pi][:], pg2[:])
                    for i in (0, 1):
                        nc.tensor.matmul(
                            acc[:], scatter_w[:, (mp + i) * N:(mp + i + 1) * N],
                            u2[:, i * f:(i + 1) * f],
                            start=(mp == 0 and i == 0), stop=False,
                            skip_group_check=True)
                # diag identity term + diag(bdo) term close the group
                # (both at row group 0 — accumulating from tile_position
                # (64,0) into an open group faults on HW)
                nc.tensor.matmul(acc[:], identb[:], diag_t[:],
                                 start=False, stop=False,
                                 skip_group_check=True)
                nc.tensor.matmul(acc[:], qdm[:], rhs[0:N, :],
                                 start=False, stop=True,
                                 skip_group_check=True)
                return acc

            def emit_tail(tt, st):
                """Both L matvecs + output for tile tt."""
                t_out = tt % ntiles
                t = f"r{tt}"
                x0T, z_sb, xd = st["x0T"], st["z"], st["xd"]
                t1 = ap.tile([N, f], BF16, tag="t1", bufs=2, name=f"t1_{t}")
                nc.vector.tensor_mul(t1[:], xd[:], x0T[0:N, :])

                py = emit_matvec(t, x0T, z_sb, rtm, cmat, t1, "y")
                y = ap.tile([128, f], BF16, tag="y", bufs=2, name=f"y{t}")
                nc.scalar.activation(y[0:N, :], py[:], Copy)
                nc.sync.dma_start(y[N:2 * N, :], y[0:N, :])
                t2 = ap.tile([N, f], BF16, tag="t2", bufs=2, name=f"t2_{t}")
                nc.vector.tensor_mul(t2[:], xd[:], y[0:N, :])

                pd = emit_matvec(t, y, z_sb, ctm, rmat, t2, "d")
                dd = ap.tile([N, f], F32R, tag="dd", bufs=2, name=f"dd{t}")
                nc.scalar.activation(dd[:], pd[:], Copy)

                # output transpose via PE into the acc ring, copy, DMA out
                po = pp.tile([128, ncol, N], F32R, tag="acc", bufs=2,
                             name=f"po{t}")
                for c in range(ncol):
                    nc.tensor.transpose(po[:, c, :],
                                        dd[:, c * 128:(c + 1) * 128],
                                        ident[:N, :N])
                o_sb = iop.tile([128, ncol, N], F32, tag="o_sb", bufs=2,
                                name=f"o_sb{t}")
                nc.scalar.activation(o_sb[:], po.rearrange("p c n -> p (c n)"),
                                     Copy)
                nc.gpsimd.dma_start(
                    out_d[t_out * f:(t_out + 1) * f, :]
                    .rearrange("(c p) n -> p c n", p=128), o_sb[:])

            # ---- flat per-tile emission ----
            for tt in range(total_tiles):
                st = emit_mlp(tt)
                emit_tail(tt, st)

    nc.compile()
    return nc


def _host_constants(Wd1, bd1, Wd2, bd2, Wdo, bdo, Wo1, bo1, Wo2, bo2, Woo, boo):
    """Shared (per-core replicated) input arrays."""
    import ml_dtypes
    f32 = np.float32
    bf16 = ml_dtypes.bfloat16
    rows, cols = np.tril_indices(N, k=-1)
    R = np.zeros((OFF, N), f32)
    R[np.arange(OFF), rows] = 1.0
    C = np.zeros((OFF, N), f32)
    C[np.arange(OFF), cols] = 1.0

    def bt(a):
        return np.ascontiguousarray(np.asarray(a, f32)).astype(bf16)

    def ct(a):
        return np.ascontiguousarray(a, dtype=f32)

    # packed scatter chunks: col block m = chunk m ([CH, N] each)
    rmpk = np.concatenate([R[m * CH:(m + 1) * CH] for m in range(NCHUNK)],
                          axis=1)
    cmpk = np.concatenate([C[m * CH:(m + 1) * CH] for m in range(NCHUNK)],
                          axis=1)
    btab = np.stack([np.asarray(b, f32).reshape(-1)
                     for b in (bo1[:128], bo1[128:], bd1[:128], bd1[128:],
                               bo2[:128], bo2[128:], bd2[:128], bd2[128:])],
                    axis=1)
    w1pk = np.vstack([np.asarray(Wo1, f32).T[:N], np.asarray(Wd1, f32).T[:N]])

    return {
        "w1pk": bt(w1pk), "wd2t": bt(np.asarray(Wd2).T),
        "wdot": bt(np.asarray(Wdo).T),
        "wo2t": bt(np.asarray(Wo2).T), "woot": bt(np.asarray(Woo).T),
        "rtmat": bt(np.vstack([R.T, R.T])), "ctmat": bt(np.vstack([C.T, C.T])),
        "rmpk": bt(rmpk), "cmpk": bt(cmpk),
        "ident": np.eye(128, dtype=f32),
        "identb": bt(np.eye(N, dtype=f32)),
        "qdm": bt(np.diag(np.asarray(bdo, f32))),
        "btab": ct(btab),
        "booc": ct(np.asarray(boo, f32).reshape(NCHUNK, CH).T),
    }


_NC_CACHE = {}


def get_nc(b_core=B_CORE, f=F, reps=1):
    key = (b_core, f, reps)
    if key not in _NC_CACHE:
        _NC_CACHE[key] = _build_nc(b_core, f, reps)
    return _NC_CACHE[key]


def make_in_maps(input, **params):
    shared = _host_constants(**params)
    x = np.ascontiguousarray(np.asarray(input), dtype=np.float32)
    assert x.shape == (B, N)
    return [dict(shared, x=x[c * B_CORE:(c + 1) * B_CORE]) for c in range(NCORES)]


def kernel(input, **params):
    from concourse import bass_utils

    nc = get_nc()
    in_maps = make_in_maps(input, **params)
    res = bass_utils.run_bass_kernel_spmd(nc, in_maps,
                                          core_ids=list(range(NCORES)))
    return np.concatenate([r["out"] for r in res.results], axis=0)
